# revision 15
# baseline (speedup 1.0000x reference)
"""Causal self-attention (B=4, T=2048, C=768, H=12) on 8 TRN2 NeuronCores.

Sharding: DP=4 over batch x TP=2 over heads (6 heads per core).
Each core computes, for its batch b and head group g:
    kqv^T projection -> K^T,Q^T per head pair [128, T]; V natural [T, 6, 65]
    (V gets a ones column so the P~@V' matmul also yields the softmax
    denominator l as row 64.)
    S^T = K^T' Q (scores transposed, k on partitions), no max subtraction
    (scores ~ N(0,1), exp is safe in fp32), causal via a -30000 triangular
    mask add + ragged matmul/exp spans.
    O~^T' = V'^T @ exp(S^T/8) accumulated over k tiles; row 64 = l.
    Normalize: r = 1/l broadcast across partitions (gpsimd), O^T = O~^T * R.
    Y_partial = O^T_stack^T @ W_proj[rows of local heads]  -> [T, 768]
Host sums the two TP partials per batch and adds b_proj.

Matmul inputs are bf16 (TensorEngine full rate); accumulation, softmax and
normalization stay fp32.
"""

import sys

sys.path.insert(0, "/opt/trn_rl_repo")

from contextlib import ExitStack

import numpy as np

import concourse.bass as bass
import concourse.tile as tile
from concourse import bacc
from concourse import mybir
from concourse.bass import ts
from concourse.bass_utils import run_bass_kernel_spmd
from concourse.masks import make_identity

F32 = mybir.dt.float32
BF16 = mybir.dt.bfloat16

B, T, C = 4, 2048, 768
H, D = 12, 64
HL = 6          # heads per core
FL = HL * D     # 384 local feature dim
NCT = C // 128  # 6 contraction tiles
NT = T // 128   # 16 token tiles
NB = T // 512   # 4 n-blocks
NPAIR = HL // 2  # 3 head pairs

MASK_NEG = -30000.0


def build_nc():
    nc = bacc.Bacc()
    x_d = nc.declare_dram_parameter("x", [T, C], F32, isOutput=False)
    wk_d = nc.declare_dram_parameter("wk", [C, FL], F32, isOutput=False)
    wq_d = nc.declare_dram_parameter("wq", [C, FL], F32, isOutput=False)
    wv_d = nc.declare_dram_parameter("wv", [C, FL], F32, isOutput=False)
    wp_d = nc.declare_dram_parameter("wp", [FL, C], F32, isOutput=False)
    bk_d = nc.declare_dram_parameter("bk", [FL], F32, isOutput=False)
    bq_d = nc.declare_dram_parameter("bq", [FL], F32, isOutput=False)
    bv_d = nc.declare_dram_parameter("bv", [FL], F32, isOutput=False)
    mask_d = nc.declare_dram_parameter("mask", [128, 128], F32, isOutput=False)
    y_d = nc.declare_dram_parameter("y", [T, C], F32, isOutput=True)

    with tile.TileContext(nc) as tc, ExitStack() as ctx:
        const = ctx.enter_context(tc.tile_pool(name="const", bufs=1))
        wpool = ctx.enter_context(tc.tile_pool(name="wpool", bufs=1))
        big = ctx.enter_context(tc.tile_pool(name="big", bufs=1))
        xtp = ctx.enter_context(tc.tile_pool(name="xtp", bufs=8))
        xin = ctx.enter_context(tc.tile_pool(name="xin", bufs=2))
        ppool = ctx.enter_context(tc.tile_pool(name="ppool", bufs=2))
        small = ctx.enter_context(tc.tile_pool(name="small", bufs=2))
        ypool = ctx.enter_context(tc.tile_pool(name="ypool", bufs=2))
        mmps = ctx.enter_context(tc.tile_pool(name="mmps", bufs=4, space="PSUM"))
        spool = ctx.enter_context(tc.tile_pool(name="spool", bufs=1, space="PSUM"))

        # ---- constants ----
        ident = const.tile([128, 128], BF16)
        make_identity(nc, ident)
        # mask[k, q] = 0 where k <= q (causal-valid), else MASK_NEG
        trimask = const.tile([128, 128], F32)
        nc.gpsimd.dma_start(out=trimask, in_=mask_d[:, :])
        ones_sb = const.tile([1, 128], BF16)
        nc.vector.memset(ones_sb, 1.0)
        bk_sb = const.tile([128, NPAIR], F32)
        bq_sb = const.tile([128, NPAIR], F32)
        nc.gpsimd.dma_start(out=bk_sb, in_=bk_d.rearrange("(i p) -> p i", p=128))
        nc.gpsimd.dma_start(out=bq_sb, in_=bq_d.rearrange("(i p) -> p i", p=128))
        # gpsimd (SWDGE) DMAs cast fp32 DRAM -> bf16 SBUF in flight
        bv_sb = const.tile([1, FL], BF16)
        nc.gpsimd.dma_start(out=bv_sb, in_=bv_d.rearrange("(o f) -> o f", o=1))

        # ---- weights: cast-DMA straight to bf16 ----
        wk_t, wq_t, wv_t, wp_t = [], [], [], []
        for ct in range(NCT):
            wkt = wpool.tile([128, FL], BF16, tag="wk", bufs=NCT, name=f"wk{ct}")
            wqt = wpool.tile([128, FL], BF16, tag="wq", bufs=NCT, name=f"wq{ct}")
            wvt = wpool.tile([128, FL], BF16, tag="wv", bufs=NCT, name=f"wv{ct}")
            nc.gpsimd.dma_start(out=wkt, in_=wk_d[ts(ct, 128), :])
            nc.gpsimd.dma_start(out=wqt, in_=wq_d[ts(ct, 128), :])
            nc.gpsimd.dma_start(out=wvt, in_=wv_d[ts(ct, 128), :])
            wk_t.append(wkt)
            wq_t.append(wqt)
            wv_t.append(wvt)
        for i in range(NPAIR):
            wpt = wpool.tile([128, C], BF16, tag="wp", bufs=NPAIR, name=f"wp{i}")
            nc.gpsimd.dma_start(out=wpt, in_=wp_d[ts(i, 128), :])
            wp_t.append(wpt)

        # ---- persistent activations ----
        kt_sb = [
            big.tile([128, T], BF16, tag="ktq", bufs=2 * NPAIR, name=f"ktp{i}")
            for i in range(NPAIR)
        ]
        qt_sb = [
            big.tile([128, T], BF16, tag="ktq", bufs=2 * NPAIR, name=f"qtp{i}")
            for i in range(NPAIR)
        ]
        v_sb = [
            big.tile([128, HL, D + 1], BF16, tag="v", bufs=NT, name=f"v{t}")
            for t in range(NT)
        ]
        otn_sb = [
            big.tile([128, T], BF16, tag="otn", bufs=NPAIR, name=f"otn{i}")
            for i in range(NPAIR)
        ]

        # ---- phase 1+2: X^T tiles, kqv^T projections, V natural ----
        for n in range(NB):
            xt_n = [
                xtp.tile([128, 512], BF16, tag="xt", name=f"xt{ct}_{n}")
                for ct in range(NCT)
            ]
            for tt in range(4):
                t = 4 * n + tt
                xb_sb = xin.tile([128, C], BF16, tag="xb", bufs=NT, name=f"xb{t}")
                nc.gpsimd.dma_start(out=xb_sb, in_=x_d[ts(t, 128), :])
                for ct in range(NCT):
                    xt_ps = mmps.tile([128, 128], BF16, tag="mm", name=f"xtps{t}_{ct}")
                    nc.tensor.transpose(
                        out=xt_ps,
                        in_=xb_sb[:, ts(ct, 128)],
                        identity=ident,
                    )
                    nc.vector.tensor_copy(out=xt_n[ct][:, ts(tt, 128)], in_=xt_ps)
            # kqv^T: K^T and Q^T pair tiles [128, T]
            for m in range(2 * NPAIR):
                w_src = wk_t if m < NPAIR else wq_t
                mi = m % NPAIR
                ps = mmps.tile([128, 512], F32, tag="mm", name=f"kqps{n}_{m}")
                for ct in range(NCT):
                    nc.tensor.matmul(
                        out=ps,
                        lhsT=w_src[ct][:, ts(mi, 128)],
                        rhs=xt_n[ct],
                        start=(ct == 0),
                        stop=(ct == NCT - 1),
                    )
                dest = kt_sb[mi] if m < NPAIR else qt_sb[mi]
                bias = (bk_sb if m < NPAIR else bq_sb)[:, mi : mi + 1]
                nc.vector.tensor_scalar_add(
                    out=dest[:, ts(n, 512)], in0=ps, scalar1=bias
                )
            # V natural (+bias via ones-row K=1 matmul)
            for tt in range(4):
                t = 4 * n + tt
                psv = mmps.tile([128, FL], F32, tag="mm", name=f"vps{t}")
                for ct in range(NCT):
                    nc.tensor.matmul(
                        out=psv,
                        lhsT=xt_n[ct][:, ts(tt, 128)],
                        rhs=wv_t[ct],
                        start=(ct == 0),
                        stop=False,
                    )
                nc.tensor.matmul(
                    out=psv,
                    lhsT=ones_sb,
                    rhs=bv_sb,
                    start=False,
                    stop=True,
                )
                nc.vector.tensor_copy(
                    out=v_sb[t][:, :, 0:D],
                    in_=psv.rearrange("p (h d) -> p h d", h=HL),
                )
                nc.vector.memset(v_sb[t][:, :, D : D + 1], 1.0)

        # ---- phase 3: attention per local head ----
        for h in range(HL):
            pair = h // 2
            row0 = 64 * (h % 2)
            kt_ap = kt_sb[pair][row0 : row0 + 64, :]
            qt_ap = qt_sb[pair][row0 : row0 + 64, :]
            sps = spool.tile([128, T], F32, tag="s", name=f"s{h}")
            otps = [
                mmps.tile([128, 512], F32, tag="mm", name=f"ot{h}_{j}")
                for j in range(NB)
            ]
            for kt in range(NT):
                c0 = 128 * kt
                for j in range(kt // 4, NB):
                    s0 = max(512 * j, c0)
                    w = 512 * (j + 1) - s0
                    nc.tensor.matmul(
                        out=sps[:, s0 : s0 + w],
                        lhsT=kt_ap[:, ts(kt, 128)],
                        rhs=qt_ap[:, s0 : s0 + w],
                        start=True,
                        stop=True,
                        tile_position=(row0, 0),
                    )
                # causal mask on the diagonal 128x128 block
                nc.vector.tensor_add(
                    sps[:, c0 : c0 + 128], sps[:, c0 : c0 + 128], trimask
                )
                pb = ppool.tile([128, T], BF16, tag="p", name=f"p{h}_{kt}")
                nc.scalar.activation(
                    out=pb[:, c0:T],
                    in_=sps[:, c0:T],
                    func=mybir.ActivationFunctionType.Exp,
                    scale=float(D) ** -0.5,
                )
                if kt % 4:
                    # stale prefix of the diagonal 512-block must read as 0
                    nc.vector.memset(pb[:, 512 * (kt // 4) : c0], 0.0)
                for j in range(kt // 4, NB):
                    s0 = max(512 * j, c0)
                    w = 512 * (j + 1) - s0
                    nc.tensor.matmul(
                        out=otps[j][0 : D + 1, s0 - 512 * j : s0 - 512 * j + w],
                        lhsT=v_sb[kt][:, h, :],
                        rhs=pb[:, s0 : s0 + w],
                        start=(kt == 0),
                        stop=(kt == 4 * j + 3),
                    )
            for j in range(NB):
                rv = small.tile([1, 512], F32, tag="r", name=f"r{h}_{j}")
                nc.vector.reciprocal(out=rv, in_=otps[j][D : D + 1, :])
                rb = small.tile([64, 512], F32, tag="R", name=f"R{h}_{j}")
                nc.gpsimd.partition_broadcast(rb, rv)
                nc.vector.tensor_mul(
                    otn_sb[pair][row0 : row0 + 64, ts(j, 512)],
                    otps[j][0:D, :],
                    rb,
                )

        # ---- phase 4: output projection (partial; host adds TP pair + bias) ----
        for qi in range(NT):
            y_sb = ypool.tile([128, C], F32, tag="y", bufs=NT, name=f"y{qi}")
            for half in range(2):
                fps = mmps.tile([128, FL], F32, tag="mm", name=f"fps{qi}_{half}")
                for pair in range(NPAIR):
                    nc.tensor.matmul(
                        out=fps,
                        lhsT=otn_sb[pair][:, ts(qi, 128)],
                        rhs=wp_t[pair][:, ts(half, FL)],
                        start=(pair == 0),
                        stop=(pair == NPAIR - 1),
                    )
                nc.vector.tensor_copy(out=y_sb[:, ts(half, FL)], in_=fps)
            nc.gpsimd.dma_start(out=y_d[ts(qi, 128), :], in_=y_sb)

    nc.compile()
    return nc


_NC = None


def _get_nc():
    global _NC
    if _NC is None:
        _NC = build_nc()
    return _NC


def make_in_maps(x, W_kqv, b_kqv, W_proj):
    ki = np.arange(128)[:, None]
    qi = np.arange(128)[None, :]
    mask = np.where(ki <= qi, 0.0, MASK_NEG).astype(np.float32)
    in_maps = []
    for core in range(8):
        b = core // 2
        h0 = (core % 2) * HL * D  # feature offset of this core's head group
        in_maps.append(
            {
                "x": np.ascontiguousarray(x[b]),
                "wk": np.ascontiguousarray(W_kqv[:, h0 : h0 + FL]),
                "wq": np.ascontiguousarray(W_kqv[:, C + h0 : C + h0 + FL]),
                "wv": np.ascontiguousarray(W_kqv[:, 2 * C + h0 : 2 * C + h0 + FL]),
                "wp": np.ascontiguousarray(W_proj[h0 : h0 + FL, :]),
                "bk": np.ascontiguousarray(b_kqv[h0 : h0 + FL]),
                "bq": np.ascontiguousarray(b_kqv[C + h0 : C + h0 + FL]),
                "bv": np.ascontiguousarray(b_kqv[2 * C + h0 : 2 * C + h0 + FL]),
                "mask": mask,
            }
        )
    return in_maps


def _combine(results, b_proj):
    y = np.empty((B, T, C), dtype=np.float32)
    for b in range(B):
        y[b] = results[2 * b]["y"] + results[2 * b + 1]["y"] + b_proj[None, :]
    return y


def kernel(x, W_kqv, b_kqv, W_proj, b_proj, **run_kwargs):
    x = np.asarray(x, dtype=np.float32)
    W_kqv = np.asarray(W_kqv, dtype=np.float32)
    b_kqv = np.asarray(b_kqv, dtype=np.float32)
    W_proj = np.asarray(W_proj, dtype=np.float32)
    b_proj = np.asarray(b_proj, dtype=np.float32)

    nc = _get_nc()
    in_maps = make_in_maps(x, W_kqv, b_kqv, W_proj)
    res = run_bass_kernel_spmd(nc, in_maps, core_ids=list(range(8)), **run_kwargs)
    out = _combine(res.results, b_proj)
    kernel.last_result = res
    return out


# revision 17
# speedup vs baseline: 1.1332x; 1.1332x over previous
"""Causal self-attention (B=4, T=2048, C=768, H=12) on 8 TRN2 NeuronCores.

Sharding: DP=4 over batch x TP=2 over heads (6 heads per core).
Each core computes, for its batch b and head group g:
    kqv^T projection -> K^T,Q^T per head pair [128, T]; V natural [T, 6, 65]
    (V gets a ones column so the P~@V' matmul also yields the softmax
    denominator l as row 64.)
    S^T = K^T' Q (scores transposed, k on partitions), no max subtraction
    (scores ~ N(0,1), exp is safe in fp32), causal via a -30000 triangular
    mask add + ragged matmul/exp spans.
    O~^T' = V'^T @ exp(S^T/8) accumulated over k tiles; row 64 = l.
    Normalize: r = 1/l broadcast across partitions (gpsimd), O^T = O~^T * R.
    Y_partial = O^T_stack^T @ W_proj[rows of local heads]  -> [T, 768]
Host sums the two TP partials per batch and adds b_proj.

Matmul inputs are bf16 (TensorEngine full rate); accumulation, softmax and
normalization stay fp32.
"""

import sys

sys.path.insert(0, "/opt/trn_rl_repo")

from contextlib import ExitStack

import numpy as np

import concourse.bass as bass
import concourse.tile as tile
from concourse import bacc
from concourse import mybir
from concourse.bass import ts
from concourse.bass_utils import run_bass_kernel_spmd
from concourse.masks import make_identity

F32 = mybir.dt.float32
BF16 = mybir.dt.bfloat16

B, T, C = 4, 2048, 768
H, D = 12, 64
HL = 6          # heads per core
FL = HL * D     # 384 local feature dim
NCT = C // 128  # 6 contraction tiles
NT = T // 128   # 16 token tiles
NB = T // 512   # 4 n-blocks
NPAIR = HL // 2  # 3 head pairs

MASK_NEG = -30000.0


def build_nc():
    nc = bacc.Bacc()
    x_d = nc.declare_dram_parameter("x", [T, C], F32, isOutput=False)
    wk_d = nc.declare_dram_parameter("wk", [C, FL], F32, isOutput=False)
    wq_d = nc.declare_dram_parameter("wq", [C, FL], F32, isOutput=False)
    wv_d = nc.declare_dram_parameter("wv", [C, FL], F32, isOutput=False)
    wp_d = nc.declare_dram_parameter("wp", [FL, C], F32, isOutput=False)
    bk_d = nc.declare_dram_parameter("bk", [FL], F32, isOutput=False)
    bq_d = nc.declare_dram_parameter("bq", [FL], F32, isOutput=False)
    bv_d = nc.declare_dram_parameter("bv", [FL], F32, isOutput=False)
    mask_d = nc.declare_dram_parameter("mask", [128, 128], F32, isOutput=False)
    y_d = nc.declare_dram_parameter("y", [T, C], F32, isOutput=True)

    with tile.TileContext(nc) as tc, ExitStack() as ctx:
        const = ctx.enter_context(tc.tile_pool(name="const", bufs=1))
        wpool = ctx.enter_context(tc.tile_pool(name="wpool", bufs=1))
        big = ctx.enter_context(tc.tile_pool(name="big", bufs=1))
        xtp = ctx.enter_context(tc.tile_pool(name="xtp", bufs=8))
        xin = ctx.enter_context(tc.tile_pool(name="xin", bufs=2))
        ppool = ctx.enter_context(tc.tile_pool(name="ppool", bufs=3))
        small = ctx.enter_context(tc.tile_pool(name="small", bufs=2))
        ypool = ctx.enter_context(tc.tile_pool(name="ypool", bufs=2))
        mmps = ctx.enter_context(tc.tile_pool(name="mmps", bufs=4, space="PSUM"))
        spool = ctx.enter_context(tc.tile_pool(name="spool", bufs=1, space="PSUM"))

        # ---- constants ----
        ident = const.tile([128, 128], BF16)
        make_identity(nc, ident)
        # mask[k, q] = 0 where k <= q (causal-valid), else MASK_NEG
        # (bf16: accumulated into scores by a PE matmul with identity lhsT)
        trimask = const.tile([128, 128], BF16)
        nc.gpsimd.dma_start(out=trimask, in_=mask_d[:, :])
        ones_sb = const.tile([1, 128], BF16)
        nc.vector.memset(ones_sb, 1.0)
        bk_sb = const.tile([128, NPAIR], F32)
        bq_sb = const.tile([128, NPAIR], F32)
        nc.gpsimd.dma_start(out=bk_sb, in_=bk_d.rearrange("(i p) -> p i", p=128))
        nc.gpsimd.dma_start(out=bq_sb, in_=bq_d.rearrange("(i p) -> p i", p=128))
        # gpsimd (SWDGE) DMAs cast fp32 DRAM -> bf16 SBUF in flight
        bv_sb = const.tile([1, FL], BF16)
        nc.gpsimd.dma_start(out=bv_sb, in_=bv_d.rearrange("(o f) -> o f", o=1))

        # ---- weights: cast-DMA straight to bf16 ----
        wk_t, wq_t, wv_t, wp_t = [], [], [], []
        for ct in range(NCT):
            wkt = wpool.tile([128, FL], BF16, tag="wk", bufs=NCT, name=f"wk{ct}")
            wqt = wpool.tile([128, FL], BF16, tag="wq", bufs=NCT, name=f"wq{ct}")
            wvt = wpool.tile([128, FL], BF16, tag="wv", bufs=NCT, name=f"wv{ct}")
            nc.gpsimd.dma_start(out=wkt, in_=wk_d[ts(ct, 128), :])
            nc.gpsimd.dma_start(out=wqt, in_=wq_d[ts(ct, 128), :])
            nc.gpsimd.dma_start(out=wvt, in_=wv_d[ts(ct, 128), :])
            wk_t.append(wkt)
            wq_t.append(wqt)
            wv_t.append(wvt)
        for i in range(NPAIR):
            wpt = wpool.tile([128, C], BF16, tag="wp", bufs=NPAIR, name=f"wp{i}")
            nc.gpsimd.dma_start(out=wpt, in_=wp_d[ts(i, 128), :])
            wp_t.append(wpt)

        # ---- persistent activations ----
        kt_sb = [
            big.tile([128, T], BF16, tag="ktq", bufs=2 * NPAIR, name=f"ktp{i}")
            for i in range(NPAIR)
        ]
        qt_sb = [
            big.tile([128, T], BF16, tag="ktq", bufs=2 * NPAIR, name=f"qtp{i}")
            for i in range(NPAIR)
        ]
        v_sb = [
            big.tile([128, HL, D + 1], BF16, tag="v", bufs=NT, name=f"v{t}")
            for t in range(NT)
        ]
        otn_sb = [
            big.tile([128, T], BF16, tag="otn", bufs=NPAIR, name=f"otn{i}")
            for i in range(NPAIR)
        ]

        # ---- phase 1+2: X^T tiles, kqv^T projections, V natural ----
        for n in range(NB):
            xt_n = [
                xtp.tile([128, 512], BF16, tag="xt", name=f"xt{ct}_{n}")
                for ct in range(NCT)
            ]
            for tt in range(4):
                t = 4 * n + tt
                xb_sb = xin.tile([128, C], BF16, tag="xb", bufs=NT, name=f"xb{t}")
                nc.gpsimd.dma_start(out=xb_sb, in_=x_d[ts(t, 128), :])
                for ct in range(NCT):
                    xt_ps = mmps.tile([128, 128], BF16, tag="mm", name=f"xtps{t}_{ct}")
                    nc.tensor.transpose(
                        out=xt_ps,
                        in_=xb_sb[:, ts(ct, 128)],
                        identity=ident,
                    )
                    nc.vector.tensor_copy(out=xt_n[ct][:, ts(tt, 128)], in_=xt_ps)
            # kqv^T: K^T and Q^T pair tiles [128, T]
            for m in range(2 * NPAIR):
                w_src = wk_t if m < NPAIR else wq_t
                mi = m % NPAIR
                ps = mmps.tile([128, 512], F32, tag="mm", name=f"kqps{n}_{m}")
                for ct in range(NCT):
                    nc.tensor.matmul(
                        out=ps,
                        lhsT=w_src[ct][:, ts(mi, 128)],
                        rhs=xt_n[ct],
                        start=(ct == 0),
                        stop=(ct == NCT - 1),
                    )
                dest = kt_sb[mi] if m < NPAIR else qt_sb[mi]
                bias = (bk_sb if m < NPAIR else bq_sb)[:, mi : mi + 1]
                nc.vector.tensor_scalar_add(
                    out=dest[:, ts(n, 512)], in0=ps, scalar1=bias
                )
            # V natural (+bias via ones-row K=1 matmul)
            for tt in range(4):
                t = 4 * n + tt
                psv = mmps.tile([128, FL], F32, tag="mm", name=f"vps{t}")
                for ct in range(NCT):
                    nc.tensor.matmul(
                        out=psv,
                        lhsT=xt_n[ct][:, ts(tt, 128)],
                        rhs=wv_t[ct],
                        start=(ct == 0),
                        stop=False,
                    )
                nc.tensor.matmul(
                    out=psv,
                    lhsT=ones_sb,
                    rhs=bv_sb,
                    start=False,
                    stop=True,
                )
                nc.vector.tensor_copy(
                    out=v_sb[t][:, :, 0:D],
                    in_=psv.rearrange("p (h d) -> p h d", h=HL),
                )
                nc.gpsimd.memset(v_sb[t][:, :, D : D + 1], 1.0)

        # ---- phase 3: attention per local head ----
        for h in range(HL):
            pair = h // 2
            row0 = 64 * (h % 2)
            kt_ap = kt_sb[pair][row0 : row0 + 64, :]
            qt_ap = qt_sb[pair][row0 : row0 + 64, :]
            sps = spool.tile([128, T], F32, tag="s", name=f"s{h}")
            otps = [
                mmps.tile([128, 512], F32, tag="mm", name=f"ot{h}_{j}")
                for j in range(NB)
            ]
            for kt in range(NT):
                c0 = 128 * kt
                for j in range(kt // 4, NB):
                    s0 = max(512 * j, c0)
                    w = 512 * (j + 1) - s0
                    nc.tensor.matmul(
                        out=sps[:, s0 : s0 + w],
                        lhsT=kt_ap[:, ts(kt, 128)],
                        rhs=qt_ap[:, s0 : s0 + w],
                        start=True,
                        stop=(j != kt // 4),
                        tile_position=(row0, 0),
                        skip_group_check=True,
                    )
                # causal mask on the diagonal 128x128 block, accumulated on
                # PE (identity^T @ trimask) to keep masking off the DVE path
                nc.tensor.matmul(
                    out=sps[:, c0 : c0 + 128],
                    lhsT=ident,
                    rhs=trimask,
                    start=False,
                    stop=True,
                    skip_group_check=True,
                )
                pb = ppool.tile([128, T], BF16, tag="p", name=f"p{h}_{kt}")
                nc.scalar.activation(
                    out=pb[:, c0:T],
                    in_=sps[:, c0:T],
                    func=mybir.ActivationFunctionType.Exp,
                    scale=float(D) ** -0.5,
                )
                if kt % 4:
                    # stale prefix of the diagonal 512-block must read as 0
                    nc.gpsimd.memset(pb[:, 512 * (kt // 4) : c0], 0.0)
                for j in range(kt // 4, NB):
                    s0 = max(512 * j, c0)
                    w = 512 * (j + 1) - s0
                    nc.tensor.matmul(
                        out=otps[j][0 : D + 1, s0 - 512 * j : s0 - 512 * j + w],
                        lhsT=v_sb[kt][:, h, :],
                        rhs=pb[:, s0 : s0 + w],
                        start=(kt == 0),
                        stop=(kt == 4 * j + 3),
                    )
            for j in range(NB):
                lv = small.tile([1, 512], F32, tag="l", name=f"l{h}_{j}")
                nc.scalar.copy(out=lv, in_=otps[j][D : D + 1, :])
                rv = small.tile([1, 512], F32, tag="r", name=f"r{h}_{j}")
                nc.vector.reciprocal_approx_fast(out=rv, in_=lv)
                rb = small.tile([64, 512], F32, tag="R", name=f"R{h}_{j}")
                nc.gpsimd.partition_broadcast(rb, rv)
                nc.vector.tensor_mul(
                    otn_sb[pair][row0 : row0 + 64, ts(j, 512)],
                    otps[j][0:D, :],
                    rb,
                )

        # ---- phase 4: output projection (partial; host adds TP pair + bias) ----
        for qi in range(NT):
            y_sb = ypool.tile([128, C], F32, tag="y", bufs=NT, name=f"y{qi}")
            for half in range(2):
                fps = mmps.tile([128, FL], F32, tag="mm", name=f"fps{qi}_{half}")
                for pair in range(NPAIR):
                    nc.tensor.matmul(
                        out=fps,
                        lhsT=otn_sb[pair][:, ts(qi, 128)],
                        rhs=wp_t[pair][:, ts(half, FL)],
                        start=(pair == 0),
                        stop=(pair == NPAIR - 1),
                    )
                nc.vector.tensor_copy(out=y_sb[:, ts(half, FL)], in_=fps)
            nc.gpsimd.dma_start(out=y_d[ts(qi, 128), :], in_=y_sb)

    nc.compile()
    return nc


_NC = None


def _get_nc():
    global _NC
    if _NC is None:
        _NC = build_nc()
    return _NC


def make_in_maps(x, W_kqv, b_kqv, W_proj):
    ki = np.arange(128)[:, None]
    qi = np.arange(128)[None, :]
    mask = np.where(ki <= qi, 0.0, MASK_NEG).astype(np.float32)
    in_maps = []
    for core in range(8):
        b = core // 2
        h0 = (core % 2) * HL * D  # feature offset of this core's head group
        in_maps.append(
            {
                "x": np.ascontiguousarray(x[b]),
                "wk": np.ascontiguousarray(W_kqv[:, h0 : h0 + FL]),
                "wq": np.ascontiguousarray(W_kqv[:, C + h0 : C + h0 + FL]),
                "wv": np.ascontiguousarray(W_kqv[:, 2 * C + h0 : 2 * C + h0 + FL]),
                "wp": np.ascontiguousarray(W_proj[h0 : h0 + FL, :]),
                "bk": np.ascontiguousarray(b_kqv[h0 : h0 + FL]),
                "bq": np.ascontiguousarray(b_kqv[C + h0 : C + h0 + FL]),
                "bv": np.ascontiguousarray(b_kqv[2 * C + h0 : 2 * C + h0 + FL]),
                "mask": mask,
            }
        )
    return in_maps


def _combine(results, b_proj):
    y = np.empty((B, T, C), dtype=np.float32)
    for b in range(B):
        y[b] = results[2 * b]["y"] + results[2 * b + 1]["y"] + b_proj[None, :]
    return y


def kernel(x, W_kqv, b_kqv, W_proj, b_proj, **run_kwargs):
    x = np.asarray(x, dtype=np.float32)
    W_kqv = np.asarray(W_kqv, dtype=np.float32)
    b_kqv = np.asarray(b_kqv, dtype=np.float32)
    W_proj = np.asarray(W_proj, dtype=np.float32)
    b_proj = np.asarray(b_proj, dtype=np.float32)

    nc = _get_nc()
    in_maps = make_in_maps(x, W_kqv, b_kqv, W_proj)
    res = run_bass_kernel_spmd(nc, in_maps, core_ids=list(range(8)), **run_kwargs)
    out = _combine(res.results, b_proj)
    kernel.last_result = res
    return out


# revision 19
# speedup vs baseline: 1.5030x; 1.3262x over previous
"""Causal self-attention (B=4, T=2048, C=768, H=12) on 8 TRN2 NeuronCores.

Sharding: DP=4 over batch x TP=2 over heads (6 heads per core).
Each core computes, for its batch b and head group g:
    kqv^T projection -> K^T,Q^T per head pair [128, T]; V natural [T, 6, 65]
    (V gets a ones column so the P~@V' matmul also yields the softmax
    denominator l as row 64.)
    S^T = K^T' Q (scores transposed, k on partitions), no max subtraction
    (scores ~ N(0,1), exp is safe in fp32), causal via a -30000 triangular
    mask add + ragged matmul/exp spans.
    O~^T' = V'^T @ exp(S^T/8) accumulated over k tiles; row 64 = l.
    Normalize: r = 1/l broadcast across partitions (gpsimd), O^T = O~^T * R.
    Y_partial = O^T_stack^T @ W_proj[rows of local heads]  -> [T, 768]
Host sums the two TP partials per batch and adds b_proj.

Matmul inputs are bf16 (TensorEngine full rate); accumulation, softmax and
normalization stay fp32.
"""

import sys

sys.path.insert(0, "/opt/trn_rl_repo")

from contextlib import ExitStack

import numpy as np

import concourse.bass as bass
import concourse.tile as tile
from concourse import bacc
from concourse import mybir
from concourse.bass import ts
from concourse.bass_utils import run_bass_kernel_spmd
from concourse.masks import make_identity

F32 = mybir.dt.float32
BF16 = mybir.dt.bfloat16

B, T, C = 4, 2048, 768
H, D = 12, 64
HL = 6          # heads per core
FL = HL * D     # 384 local feature dim
NCT = C // 128  # 6 contraction tiles
NT = T // 128   # 16 token tiles
NB = T // 512   # 4 n-blocks
NPAIR = HL // 2  # 3 head pairs

MASK_NEG = -30000.0


def build_nc():
    nc = bacc.Bacc()
    x_d = nc.declare_dram_parameter("x", [T, C], F32, isOutput=False)
    wk_d = nc.declare_dram_parameter("wk", [C, FL], F32, isOutput=False)
    wq_d = nc.declare_dram_parameter("wq", [C, FL], F32, isOutput=False)
    wv_d = nc.declare_dram_parameter("wv", [C, FL], F32, isOutput=False)
    wp_d = nc.declare_dram_parameter("wp", [FL, C], F32, isOutput=False)
    bk_d = nc.declare_dram_parameter("bk", [FL], F32, isOutput=False)
    bq_d = nc.declare_dram_parameter("bq", [FL], F32, isOutput=False)
    bv_d = nc.declare_dram_parameter("bv", [FL], F32, isOutput=False)
    mask_d = nc.declare_dram_parameter("mask", [128, 128], F32, isOutput=False)
    y_d = nc.declare_dram_parameter("y", [T, C], F32, isOutput=True)

    with tile.TileContext(nc) as tc, ExitStack() as ctx:
        const = ctx.enter_context(tc.tile_pool(name="const", bufs=1))
        wpool = ctx.enter_context(tc.tile_pool(name="wpool", bufs=1))
        big = ctx.enter_context(tc.tile_pool(name="big", bufs=1))
        xtp = ctx.enter_context(tc.tile_pool(name="xtp", bufs=8))
        xin = ctx.enter_context(tc.tile_pool(name="xin", bufs=2))
        ppool = ctx.enter_context(tc.tile_pool(name="ppool", bufs=4))
        small = ctx.enter_context(tc.tile_pool(name="small", bufs=2))
        ypool = ctx.enter_context(tc.tile_pool(name="ypool", bufs=2))
        mmps = ctx.enter_context(tc.tile_pool(name="mmps", bufs=4, space="PSUM"))
        spool = ctx.enter_context(tc.tile_pool(name="spool", bufs=2, space="PSUM"))

        # ---- constants ----
        ident = const.tile([128, 128], BF16)
        make_identity(nc, ident)
        # mask[k, q] = 0 where k <= q (causal-valid), else MASK_NEG
        # (bf16: accumulated into scores by a PE matmul with identity lhsT)
        trimask = const.tile([128, 128], BF16)
        nc.gpsimd.dma_start(out=trimask, in_=mask_d[:, :])
        ones_sb = const.tile([1, 128], BF16)
        nc.vector.memset(ones_sb, 1.0)
        bk_sb = const.tile([128, NPAIR], F32)
        bq_sb = const.tile([128, NPAIR], F32)
        nc.gpsimd.dma_start(out=bk_sb, in_=bk_d.rearrange("(i p) -> p i", p=128))
        nc.gpsimd.dma_start(out=bq_sb, in_=bq_d.rearrange("(i p) -> p i", p=128))
        # gpsimd (SWDGE) DMAs cast fp32 DRAM -> bf16 SBUF in flight
        bv_sb = const.tile([1, FL], BF16)
        nc.gpsimd.dma_start(out=bv_sb, in_=bv_d.rearrange("(o f) -> o f", o=1))

        # ---- weights: cast-DMA straight to bf16 ----
        wk_t, wq_t, wv_t, wp_t = [], [], [], []
        for ct in range(NCT):
            wkt = wpool.tile([128, FL], BF16, tag="wk", bufs=NCT, name=f"wk{ct}")
            wqt = wpool.tile([128, FL], BF16, tag="wq", bufs=NCT, name=f"wq{ct}")
            wvt = wpool.tile([128, FL], BF16, tag="wv", bufs=NCT, name=f"wv{ct}")
            nc.gpsimd.dma_start(out=wkt, in_=wk_d[ts(ct, 128), :])
            nc.gpsimd.dma_start(out=wqt, in_=wq_d[ts(ct, 128), :])
            nc.gpsimd.dma_start(out=wvt, in_=wv_d[ts(ct, 128), :])
            wk_t.append(wkt)
            wq_t.append(wqt)
            wv_t.append(wvt)
        for i in range(NPAIR):
            wpt = wpool.tile([128, C], BF16, tag="wp", bufs=NPAIR, name=f"wp{i}")
            nc.gpsimd.dma_start(out=wpt, in_=wp_d[ts(i, 128), :])
            wp_t.append(wpt)

        # ---- persistent activations ----
        kt_sb = [
            big.tile([128, T], BF16, tag="ktq", bufs=2 * NPAIR, name=f"ktp{i}")
            for i in range(NPAIR)
        ]
        qt_sb = [
            big.tile([128, T], BF16, tag="ktq", bufs=2 * NPAIR, name=f"qtp{i}")
            for i in range(NPAIR)
        ]
        v_sb = [
            big.tile([128, HL, D + 1], BF16, tag="v", bufs=NT, name=f"v{t}")
            for t in range(NT)
        ]
        otn_sb = [
            big.tile([128, T], BF16, tag="otn", bufs=NPAIR, name=f"otn{i}")
            for i in range(NPAIR)
        ]

        # ---- phase 1+2: X^T tiles, kqv^T projections, V natural ----
        for n in range(NB):
            xt_n = [
                xtp.tile([128, 512], BF16, tag="xt", name=f"xt{ct}_{n}")
                for ct in range(NCT)
            ]
            for tt in range(4):
                t = 4 * n + tt
                xb_sb = xin.tile([128, C], BF16, tag="xb", bufs=NT, name=f"xb{t}")
                nc.gpsimd.dma_start(out=xb_sb, in_=x_d[ts(t, 128), :])
                for ct in range(NCT):
                    xt_ps = mmps.tile([128, 128], BF16, tag="mm", name=f"xtps{t}_{ct}")
                    nc.tensor.transpose(
                        out=xt_ps,
                        in_=xb_sb[:, ts(ct, 128)],
                        identity=ident,
                    )
                    nc.vector.tensor_copy(out=xt_n[ct][:, ts(tt, 128)], in_=xt_ps)
            # kqv^T: K^T and Q^T pair tiles [128, T]
            for m in range(2 * NPAIR):
                w_src = wk_t if m < NPAIR else wq_t
                mi = m % NPAIR
                ps = mmps.tile([128, 512], F32, tag="mm", name=f"kqps{n}_{m}")
                for ct in range(NCT):
                    nc.tensor.matmul(
                        out=ps,
                        lhsT=w_src[ct][:, ts(mi, 128)],
                        rhs=xt_n[ct],
                        start=(ct == 0),
                        stop=(ct == NCT - 1),
                    )
                dest = kt_sb[mi] if m < NPAIR else qt_sb[mi]
                bias = (bk_sb if m < NPAIR else bq_sb)[:, mi : mi + 1]
                nc.vector.tensor_scalar_add(
                    out=dest[:, ts(n, 512)], in0=ps, scalar1=bias
                )
            # V natural (+bias via ones-row K=1 matmul)
            for tt in range(4):
                t = 4 * n + tt
                psv = mmps.tile([128, FL], F32, tag="mm", name=f"vps{t}")
                for ct in range(NCT):
                    nc.tensor.matmul(
                        out=psv,
                        lhsT=xt_n[ct][:, ts(tt, 128)],
                        rhs=wv_t[ct],
                        start=(ct == 0),
                        stop=False,
                    )
                nc.tensor.matmul(
                    out=psv,
                    lhsT=ones_sb,
                    rhs=bv_sb,
                    start=False,
                    stop=True,
                )
                nc.vector.tensor_copy(
                    out=v_sb[t][:, :, 0:D],
                    in_=psv.rearrange("p (h d) -> p h d", h=HL),
                )
                nc.gpsimd.memset(v_sb[t][:, :, D : D + 1], 1.0)

        # ---- phase 3: attention, two heads of a pair interleaved ----
        # Each (pair, J) works on a 1024-wide query half; the two heads get
        # independent S / OT psum tiles so the scheduler can run one head's
        # matmuls while the other head's exp is on the scalar engine.
        for pair in range(NPAIR):
            for J in range(2):
                hs = (2 * pair, 2 * pair + 1)
                q0 = 1024 * J
                sps_h = {}
                ot_h = {}
                pb_h = {}
                for h in hs:
                    sps_h[h] = spool.tile(
                        [128, 1024], F32, tag="s", name=f"s{h}_{J}"
                    )
                    ot_h[h] = {
                        j: mmps.tile(
                            [128, 512], F32, tag="mm", name=f"ot{h}_{j}"
                        )
                        for j in (2 * J, 2 * J + 1)
                    }
                for kt in range(8 * J + 8):
                    c0 = 128 * kt
                    diag = 8 * J <= kt  # diagonal block lands in this half
                    for h in hs:
                        row0 = 64 * (h % 2)
                        kt_ap = kt_sb[pair][row0 : row0 + 64, :]
                        qt_ap = qt_sb[pair][row0 : row0 + 64, :]
                        sps = sps_h[h]
                        for j in range(max(2 * J, kt // 4), 2 * J + 2):
                            s0 = max(512 * j, c0)
                            w = 512 * (j + 1) - s0
                            nc.tensor.matmul(
                                out=sps[:, s0 - q0 : s0 - q0 + w],
                                lhsT=kt_ap[:, ts(kt, 128)],
                                rhs=qt_ap[:, s0 : s0 + w],
                                start=True,
                                stop=not (diag and j == kt // 4),
                                tile_position=(row0, 0),
                                skip_group_check=True,
                            )
                        if diag:
                            # causal mask on the diagonal 128x128 block, done
                            # on PE (identity^T @ trimask) to keep it off DVE
                            nc.tensor.matmul(
                                out=sps[:, c0 - q0 : c0 - q0 + 128],
                                lhsT=ident,
                                rhs=trimask,
                                start=False,
                                stop=True,
                                skip_group_check=True,
                            )
                        e0 = max(q0, c0)
                        pb = ppool.tile(
                            [128, 1024], BF16, tag="p", name=f"p{h}_{kt}_{J}"
                        )
                        nc.scalar.activation(
                            out=pb[:, e0 - q0 : 1024],
                            in_=sps[:, e0 - q0 : 1024],
                            func=mybir.ActivationFunctionType.Exp,
                            scale=float(D) ** -0.5,
                        )
                        if diag and kt % 4:
                            # stale prefix of the diagonal 512-block -> 0
                            nc.gpsimd.memset(
                                pb[:, 512 * (kt // 4) - q0 : c0 - q0], 0.0
                            )
                        for j in range(max(2 * J, kt // 4), 2 * J + 2):
                            s0 = max(512 * j, c0)
                            w = 512 * (j + 1) - s0
                            nc.tensor.matmul(
                                out=ot_h[h][j][
                                    0 : D + 1, s0 - 512 * j : s0 - 512 * j + w
                                ],
                                lhsT=v_sb[kt][:, h, :],
                                rhs=pb[:, s0 - q0 : s0 - q0 + w],
                                start=(kt == 0),
                                stop=(kt == 4 * j + 3),
                            )
                for h in hs:
                    row0 = 64 * (h % 2)
                    for j in (2 * J, 2 * J + 1):
                        otps = ot_h[h][j]
                        lv = small.tile([1, 512], F32, tag="l", name=f"l{h}_{j}")
                        nc.scalar.copy(out=lv, in_=otps[D : D + 1, :])
                        rv = small.tile([1, 512], F32, tag="r", name=f"r{h}_{j}")
                        nc.vector.reciprocal_approx_fast(out=rv, in_=lv)
                        rb = small.tile([64, 512], F32, tag="R", name=f"R{h}_{j}")
                        nc.gpsimd.partition_broadcast(rb, rv)
                        nc.vector.tensor_mul(
                            otn_sb[pair][row0 : row0 + 64, ts(j, 512)],
                            otps[0:D, :],
                            rb,
                        )

        # ---- phase 4: output projection (partial; host adds TP pair + bias) ----
        for qi in range(NT):
            y_sb = ypool.tile([128, C], F32, tag="y", bufs=NT, name=f"y{qi}")
            for half in range(2):
                fps = mmps.tile([128, FL], F32, tag="mm", name=f"fps{qi}_{half}")
                for pair in range(NPAIR):
                    nc.tensor.matmul(
                        out=fps,
                        lhsT=otn_sb[pair][:, ts(qi, 128)],
                        rhs=wp_t[pair][:, ts(half, FL)],
                        start=(pair == 0),
                        stop=(pair == NPAIR - 1),
                    )
                nc.vector.tensor_copy(out=y_sb[:, ts(half, FL)], in_=fps)
            nc.gpsimd.dma_start(out=y_d[ts(qi, 128), :], in_=y_sb)

    nc.compile()
    return nc


_NC = None


def _get_nc():
    global _NC
    if _NC is None:
        _NC = build_nc()
    return _NC


def make_in_maps(x, W_kqv, b_kqv, W_proj):
    ki = np.arange(128)[:, None]
    qi = np.arange(128)[None, :]
    mask = np.where(ki <= qi, 0.0, MASK_NEG).astype(np.float32)
    in_maps = []
    for core in range(8):
        b = core // 2
        h0 = (core % 2) * HL * D  # feature offset of this core's head group
        in_maps.append(
            {
                "x": np.ascontiguousarray(x[b]),
                "wk": np.ascontiguousarray(W_kqv[:, h0 : h0 + FL]),
                "wq": np.ascontiguousarray(W_kqv[:, C + h0 : C + h0 + FL]),
                "wv": np.ascontiguousarray(W_kqv[:, 2 * C + h0 : 2 * C + h0 + FL]),
                "wp": np.ascontiguousarray(W_proj[h0 : h0 + FL, :]),
                "bk": np.ascontiguousarray(b_kqv[h0 : h0 + FL]),
                "bq": np.ascontiguousarray(b_kqv[C + h0 : C + h0 + FL]),
                "bv": np.ascontiguousarray(b_kqv[2 * C + h0 : 2 * C + h0 + FL]),
                "mask": mask,
            }
        )
    return in_maps


def _combine(results, b_proj):
    y = np.empty((B, T, C), dtype=np.float32)
    for b in range(B):
        y[b] = results[2 * b]["y"] + results[2 * b + 1]["y"] + b_proj[None, :]
    return y


def kernel(x, W_kqv, b_kqv, W_proj, b_proj, **run_kwargs):
    x = np.asarray(x, dtype=np.float32)
    W_kqv = np.asarray(W_kqv, dtype=np.float32)
    b_kqv = np.asarray(b_kqv, dtype=np.float32)
    W_proj = np.asarray(W_proj, dtype=np.float32)
    b_proj = np.asarray(b_proj, dtype=np.float32)

    nc = _get_nc()
    in_maps = make_in_maps(x, W_kqv, b_kqv, W_proj)
    res = run_bass_kernel_spmd(nc, in_maps, core_ids=list(range(8)), **run_kwargs)
    out = _combine(res.results, b_proj)
    kernel.last_result = res
    return out
